# revision 1
# baseline (speedup 1.0000x reference)
"""Trainium2 Bass kernel for nn_Attention_80779744903968.

Reference computation (B=32, T=512, S=1024, H=1024):
    z      = q @ W_in.T                  [B,T,H]
    scores = z @ enc_b.T                 [B,T,S]   (enc input is [S,B,H])
    p      = softmax(scores, axis=-1)    (the scores==0 -> -inf fill is a
                                          numerical no-op: row maxes are ~120,
                                          exp(0-max) == 0 in fp32)
    c      = p @ enc_b                   [B,T,H]
    out    = tanh([c, q] @ W_out.T + b)  [B,T,H]

Sharding: data-parallel over B across 8 cores (4 batches per core).
W_in / W_out replicated.

Precision strategy (PE matmuls):
  - z and scores matmuls use an fp16 hi/lo 3-pass split
    (x = xh + xl, x*y ~= xh*yh + xh*yl + xl*yh, fp32 PSUM accumulation),
    giving ~fp32-quality logits at 3x bf16-rate cost. Needed because the
    softmax is near-one-hot (logit std ~37) with some near-tied rows.
  - downstream (p, enc, c, q, W_out) runs in plain fp16: p is in [0,1] and
    c/out magnitudes are O(1), so fp16's 2^-11 relative error is plenty.

All transposes (q -> [H,T], enc -> [H,S] per batch) are done on the host so
every device-side DMA is a contiguous natural-layout load; only the softmax
output p is transposed on-device (PE transpose-mode, fp16, 128x128 tiles).
"""
import os
import sys

import numpy as np

sys.path.insert(0, "/opt/trn_rl_repo")

import concourse.bass as bass  # noqa: E402
import concourse.tile as tile  # noqa: E402
from concourse import bacc, mybir  # noqa: E402
from concourse.bass_utils import run_bass_kernel_spmd  # noqa: E402
from concourse.masks import make_identity  # noqa: E402

B, T, S, H = 32, 512, 1024, 1024
NCORES = 8
BL = B // NCORES  # batches per core
HT = H // 128     # h/i/k tiles per 1024
TT = T // 128     # t tiles
ST = S // 128     # s tiles
F16 = mybir.dt.float16
F32 = mybir.dt.float32

_CACHE = {}


def _build():
    nc = bacc.Bacc("TRN2", target_bir_lowering=False, debug=False,
                   num_devices=NCORES)

    def din(name, shape, dt=F16):
        return nc.dram_tensor(name, shape, dt, kind="ExternalInput").ap()

    qh_d = din("qh", [BL, H, T])
    ql_d = din("ql", [BL, H, T])
    eh_d = din("eh", [BL, H, S])
    el_d = din("el", [BL, H, S])
    en_d = din("en", [BL, S, H])
    wh_d = din("wh", [H, H])
    wl_d = din("wl", [H, H])
    wo_d = din("wo", [2 * H, H])
    bias_d = din("bias", [128, H], F32)
    out_d = nc.dram_tensor("out", [BL, T, H], F32, kind="ExternalOutput").ap()

    with tile.TileContext(nc) as tc:
        with (
            tc.tile_pool(name="weights", bufs=1) as wp,
            tc.tile_pool(name="qin", bufs=2) as qp,
            tc.tile_pool(name="ein", bufs=1) as ep,
            tc.tile_pool(name="enin", bufs=1) as enp,
            tc.tile_pool(name="zbuf", bufs=1) as zp,
            tc.tile_pool(name="scores", bufs=2) as scp,
            tc.tile_pool(name="pbuf", bufs=2) as pp,
            tc.tile_pool(name="ptbuf", bufs=1) as ptp,
            tc.tile_pool(name="ctbuf", bufs=1) as ctp,
            tc.tile_pool(name="ostage", bufs=4) as op,
            tc.tile_pool(name="stats", bufs=8) as stp,
            tc.tile_pool(name="psmm", bufs=4, space="PSUM") as psmm,
            tc.tile_pool(name="pstr", bufs=4, space="PSUM") as pstr,
        ):
            # --- resident weights / constants ---
            wh_t = wp.tile([128, HT, H], F16)
            nc.sync.dma_start(wh_t[:], wh_d.rearrange("(ht p) i -> p ht i", p=128))
            wl_t = wp.tile([128, HT, H], F16)
            nc.sync.dma_start(wl_t[:], wl_d.rearrange("(ht p) i -> p ht i", p=128))
            wo_t = wp.tile([128, 2 * HT, H], F16)
            nc.sync.dma_start(wo_t[:], wo_d.rearrange("(kt p) h -> p kt h", p=128))
            bias_t = wp.tile([128, H], F32)
            nc.sync.dma_start(bias_t[:], bias_d)
            ident = wp.tile([128, 128], F16)
            make_identity(nc, ident[:])

            for b in range(BL):
                qh_t = qp.tile([128, HT, T], F16, tag="qh")
                nc.sync.dma_start(
                    qh_t[:], qh_d[b].rearrange("(ht p) t -> p ht t", p=128))
                ql_t = qp.tile([128, HT, T], F16, tag="ql")
                nc.sync.dma_start(
                    ql_t[:], ql_d[b].rearrange("(ht p) t -> p ht t", p=128))
                eh_t = ep.tile([128, HT, S], F16, tag="eh")
                nc.sync.dma_start(
                    eh_t[:], eh_d[b].rearrange("(it p) s -> p it s", p=128))
                el_t = ep.tile([128, HT, S], F16, tag="el")
                nc.sync.dma_start(
                    el_t[:], el_d[b].rearrange("(it p) s -> p it s", p=128))
                en_t = enp.tile([128, ST, H], F16, tag="en")
                nc.sync.dma_start(
                    en_t[:], en_d[b].rearrange("(st p) k -> p st k", p=128))

                # --- z^T = W_inT.T @ qT  (hi/lo 3-pass) -> zh/zl [i, t] f16 ---
                zh_t = zp.tile([128, HT, T], F16, tag="zh")
                zl_t = zp.tile([128, HT, T], F16, tag="zl")
                for it in range(HT):
                    zps = psmm.tile([128, T], F32, tag="mm")
                    n_mm = 3 * HT
                    j = 0
                    for lhs, rhs in ((wh_t, qh_t), (wh_t, ql_t), (wl_t, qh_t)):
                        for ht in range(HT):
                            nc.tensor.matmul(
                                zps[:],
                                lhs[:, ht, it * 128:(it + 1) * 128],
                                rhs[:, ht, :],
                                start=(j == 0), stop=(j == n_mm - 1),
                            )
                            j += 1
                    nc.vector.tensor_copy(zh_t[:, it, :], zps[:])
                    nc.vector.tensor_sub(zl_t[:, it, :], zps[:], zh_t[:, it, :])

                # --- scores = zT.T @ encT (hi/lo 3-pass) + softmax -> p ---
                p_tiles = []
                for tt in range(TT):
                    sc_t = scp.tile([128, S], F32, tag="sc")
                    for sc in range(2):
                        sps = psmm.tile([128, 512], F32, tag="mm")
                        n_mm = 3 * HT
                        j = 0
                        for lhs, rhs in ((zh_t, eh_t), (zh_t, el_t),
                                         (zl_t, eh_t)):
                            for it in range(HT):
                                nc.tensor.matmul(
                                    sps[:],
                                    lhs[:, it, tt * 128:(tt + 1) * 128],
                                    rhs[:, it, sc * 512:(sc + 1) * 512],
                                    start=(j == 0), stop=(j == n_mm - 1),
                                )
                                j += 1
                        nc.vector.tensor_copy(sc_t[:, sc * 512:(sc + 1) * 512],
                                              sps[:])
                    # softmax over free dim (s)
                    negmax = stp.tile([128, 1], F32, tag="nm")
                    nc.vector.reduce_max(out=negmax[:], in_=sc_t[:],
                                         axis=mybir.AxisListType.X, negate=True)
                    p_t = pp.tile([128, S], F16, tag="p")
                    nc.scalar.activation(
                        out=p_t[:], in_=sc_t[:],
                        func=mybir.ActivationFunctionType.Exp,
                        bias=negmax[:], scale=1.0,
                    )
                    ssum = stp.tile([128, 1], F32, tag="ss")
                    nc.vector.reduce_sum(out=ssum[:], in_=p_t[:],
                                         axis=mybir.AxisListType.X)
                    rsum = stp.tile([128, 1], F32, tag="rs")
                    nc.vector.reciprocal(rsum[:], ssum[:])
                    nc.vector.tensor_scalar_mul(p_t[:], p_t[:], rsum[:])
                    p_tiles.append(p_t)

                # --- transpose p -> pT [s, t] f16 (PE transpose 128x128) ---
                pt_t = ptp.tile([128, ST, T], F16, tag="pt")
                for tt in range(TT):
                    for st in range(ST):
                        tps = pstr.tile([128, 128], F16, tag="tr")
                        nc.tensor.transpose(
                            tps[:], p_tiles[tt][:, st * 128:(st + 1) * 128],
                            ident[:])
                        nc.vector.tensor_copy(
                            pt_t[:, st, tt * 128:(tt + 1) * 128], tps[:])

                # --- cT = enc_nat.T @ pT -> [k, t] f16 ---
                ct_t = ctp.tile([128, HT, T], F16, tag="ct")
                for kt in range(HT):
                    cps = psmm.tile([128, T], F32, tag="mm")
                    for st in range(ST):
                        nc.tensor.matmul(
                            cps[:],
                            en_t[:, st, kt * 128:(kt + 1) * 128],
                            pt_t[:, st, :],
                            start=(st == 0), stop=(st == ST - 1),
                        )
                    nc.vector.tensor_copy(ct_t[:, kt, :], cps[:])

                # --- out = tanh(cT.T @ WcT + qT.T @ WqT + b) ---
                for tt in range(TT):
                    for hc in range(2):
                        ops = psmm.tile([128, 512], F32, tag="mm")
                        j = 0
                        for kt in range(HT):
                            nc.tensor.matmul(
                                ops[:],
                                ct_t[:, kt, tt * 128:(tt + 1) * 128],
                                wo_t[:, kt, hc * 512:(hc + 1) * 512],
                                start=(j == 0), stop=False,
                            )
                            j += 1
                        for ht in range(HT):
                            nc.tensor.matmul(
                                ops[:],
                                qh_t[:, ht, tt * 128:(tt + 1) * 128],
                                wo_t[:, HT + ht, hc * 512:(hc + 1) * 512],
                                start=False, stop=(ht == HT - 1),
                            )
                        ost = op.tile([128, 512], F32, tag="os")
                        nc.vector.tensor_add(
                            ost[:], ops[:], bias_t[:, hc * 512:(hc + 1) * 512])
                        nc.scalar.activation(
                            out=ost[:], in_=ost[:],
                            func=mybir.ActivationFunctionType.Tanh)
                        nc.sync.dma_start(
                            out_d[b, tt * 128:(tt + 1) * 128,
                                  hc * 512:(hc + 1) * 512],
                            ost[:])

    nc.compile()
    return nc


def _get_nc():
    if "nc" not in _CACHE:
        _CACHE["nc"] = _build()
    return _CACHE["nc"]


def _split16(x):
    hi = x.astype(np.float16)
    lo = (x - hi.astype(np.float32)).astype(np.float16)
    return hi, lo


def kernel(query, encoder_outputs, src_lengths, W_in, W_out, b_out):
    query = np.asarray(query, np.float32)
    enc = np.asarray(encoder_outputs, np.float32)
    W_in = np.asarray(W_in, np.float32)
    W_out = np.asarray(W_out, np.float32)
    b_out = np.asarray(b_out, np.float32)

    # host-side layout prep (transposes + fp16 hi/lo splits)
    qT = np.ascontiguousarray(query.transpose(0, 2, 1))        # [B, H, T]
    qh, ql = _split16(qT)
    encT = np.ascontiguousarray(enc.transpose(1, 2, 0))        # [B, H, S]
    eh, el = _split16(encT)
    en = np.ascontiguousarray(enc.transpose(1, 0, 2)).astype(np.float16)
    wh, wl = _split16(np.ascontiguousarray(W_in.T))            # [H(h), H(i)]
    wo = np.ascontiguousarray(W_out.T).astype(np.float16)      # [2H, H]
    bias = np.broadcast_to(b_out[None, :], (128, H))
    bias = np.ascontiguousarray(bias, np.float32)

    in_maps = []
    for c in range(NCORES):
        sl = slice(c * BL, (c + 1) * BL)
        in_maps.append({
            "qh": np.ascontiguousarray(qh[sl]),
            "ql": np.ascontiguousarray(ql[sl]),
            "eh": np.ascontiguousarray(eh[sl]),
            "el": np.ascontiguousarray(el[sl]),
            "en": np.ascontiguousarray(en[sl]),
            "wh": wh, "wl": wl, "wo": wo, "bias": bias,
        })

    nc = _get_nc()
    trace = bool(int(os.environ.get("KERNEL_TRACE", "0")))
    res = run_bass_kernel_spmd(nc, in_maps, core_ids=list(range(NCORES)),
                               trace=trace)
    if trace:
        _CACHE["last_exec_time_ns"] = res.exec_time_ns
        _CACHE["last_results"] = res
    out = np.concatenate([r["out"] for r in res.results], axis=0)
    return out
